# revision 1
# baseline (speedup 1.0000x reference)
"""Binary-tree gated-expert MoE kernel for 8 Trainium2 NeuronCores.

Reference computation (B=4096, D=2048, 4 levels, 1/2/4/8 experts):
    h = x
    for level l: h = relu(h @ Wl[eid_l] + bl[eid_l])
where eid_l is the l-bit prefix of the 3-bit leaf id built from
path_mask[:, 0:3].

Strategy: expert-parallel over the 8 leaves with host-side dispatch.
Sorting samples by leaf id makes every level's expert groups contiguous
(level-l ids are prefixes of the leaf id), so core c processes leaf
group c and needs exactly 4 weight matrices: W0[0], W1[c>>2], W2[c>>1],
W3[c].  Groups are Binomial(B, 1/8) ~ 512+-21 rows; each is padded to a
common per-core batch Bc.  On device each core runs 4 chained
matmul+relu levels in feature-major layout (activations stay transposed
[D, Bc] in SBUF across all levels; output partition dim = output
features, so no transposes anywhere).  Matmuls run in float32r (full PE
rate, ~1.6e-4 matmul rel-err).  Weights stream HBM->SBUF per 512-column
group, double buffered.
"""

import math

import numpy as np

from concourse import bacc, mybir, tile
from concourse.bass_utils import run_bass_kernel_spmd

D = 2048
KT = D // 128          # 16 contraction k-tiles
JT = D // 128          # 16 output-feature blocks
JG = 4                 # j-groups of 4 blocks (512 features) per W DMA
N_CORES = 8
N_LEVELS = 4
F32 = mybir.dt.float32
F32R = mybir.dt.float32r

_cache: dict = {}


def _build(Bc: int, chunk: int, nchunks: int):
    """Build + compile the per-core Bass program for batch Bc = chunk*nchunks."""
    key = (Bc, chunk, nchunks)
    if key in _cache:
        return _cache[key]

    nc = bacc.Bacc("TRN2", target_bir_lowering=False, debug=False,
                   num_devices=N_CORES)

    # Weights arrive host-linearized as [JG, 128, KT*512]:
    # element (jg, p, kt, jc) = W[kt*128 + p, jg*512 + jc], so each DMA
    # reads long contiguous runs per partition.
    xT = nc.dram_tensor("xT", [D, Bc], F32, kind="ExternalInput")
    Ws = [nc.dram_tensor(f"W{l}", [JG, 128, KT * 512], F32,
                         kind="ExternalInput")
          for l in range(N_LEVELS)]
    bias = nc.dram_tensor("bias", [N_LEVELS, D], F32, kind="ExternalInput")
    out = nc.dram_tensor("out", [D, Bc], F32, kind="ExternalOutput")

    xTv = xT.rearrange("(kt p) b -> p kt b", p=128).bitcast(F32R)
    outv = out.rearrange("(jt p) b -> p jt b", p=128)
    bv = bias.rearrange("l (jt p) -> p l jt", p=128)
    NQ = 4                      # W DMA split: 4 quarters of 4 k-tiles
    KQ = KT // NQ               # k-tiles per quarter
    QW = KQ * 512               # W free-dim elements per quarter
    PACE_WIN = 3                # max in-flight paced DMAs on the SP ring

    with tile.TileContext(nc) as tc:
        with (
            tc.tile_pool(name="acts", bufs=1) as acts,
            tc.tile_pool(name="w", bufs=3) as wpool,
            tc.tile_pool(name="ps", bufs=8, space="PSUM") as ps,
            tc.tile_pool(name="misc", bufs=1) as misc,
        ):
            actA = acts.tile([128, KT, Bc], F32R, tag="A")
            actB = acts.tile([128, KT, Bc], F32R, tag="B")
            btile = misc.tile([128, N_LEVELS, JT], F32)
            nc.scalar.dma_start(btile[:], bv)

            # Warm the PE HAM clock gate during the DMA lead-in: ~6us of
            # throwaway matmuls on a zeroed tile so the first real matmul
            # runs at 2.4GHz instead of 1.2GHz.
            warm = misc.tile([128, 512], mybir.dt.bfloat16)
            nc.gpsimd.memset(warm[:], 0.0)
            wacc = ps.tile([128, 512], F32, tag="ps", name="wacc")
            for _ in range(30):
                nc.tensor.matmul(wacc[:], warm[:, :128], warm[:],
                                 start=True, stop=True)

            # All bulk input DMAs go on the SP ring, chained so at most
            # PACE_WIN are in flight.  The HW SDMA engines round-robin
            # packets across every queued transfer, so an unbounded
            # backlog makes every transfer finish near the end; a short
            # chain keeps completion order = consumption order with the
            # stream still running at full HBM rate.
            paced = []

            def paced_dma(dst_ap, src_ap):
                h = nc.sync.dma_start(dst_ap, src_ap)
                # tighter window while the first matmul's inputs stream
                win = 2 if len(paced) < 4 else PACE_WIN
                if len(paced) >= win:
                    tile.add_dep_helper(h.ins, paced[-win].ins,
                                        reason="dma pacing chain")
                paced.append(h)
                return h

            # x pieces, emitted interleaved with the first weight
            # quarters in consumption-priority order.  The first piece
            # (k-quarter 0, chunk 0) is all the first matmul needs.
            pend_x = []
            if nchunks > 1:
                pend_x.append((slice(0, KQ), slice(chunk, Bc)))
            for q in range(1, NQ):
                pend_x.append((slice(q * KQ, (q + 1) * KQ), slice(0, Bc)))
            paced_dma(actA[:, 0:KQ, 0:chunk], xTv[:, 0:KQ, 0:chunk])

            for l in range(N_LEVELS):
                src = actA if l % 2 == 0 else actB
                dst = actB if l % 2 == 0 else actA
                for jg in range(JG):
                    wt = wpool.tile([128, KT, 4 * 128], F32R, tag="w")
                    wflat = wt.rearrange("p kt j -> p (kt j)")
                    accs = [ps.tile([128, chunk], F32, tag="ps", name="acc")
                            for _ in range(nchunks * 4)]
                    for q in range(NQ):
                        paced_dma(
                            wflat[:, q * QW:(q + 1) * QW],
                            Ws[l][jg][:, q * QW:(q + 1) * QW].bitcast(F32R))
                        if l == 0 and jg == 0 and pend_x:
                            ks, cs_x = pend_x.pop(0)
                            paced_dma(actA[:, ks, cs_x], xTv[:, ks, cs_x])
                        for c in range(nchunks):
                            cs = slice(c * chunk, (c + 1) * chunk)
                            for jj in range(4):
                                acc = accs[c * 4 + jj]
                                for kt in range(q * KQ, (q + 1) * KQ):
                                    nc.tensor.matmul(
                                        acc[:],
                                        wt[:, kt, jj * 128:(jj + 1) * 128],
                                        src[:, kt, cs],
                                        start=(kt == 0),
                                        stop=(kt == KT - 1),
                                    )
                    for c in range(nchunks):
                        cs = slice(c * chunk, (c + 1) * chunk)
                        for jj in range(4):
                            jt = jg * 4 + jj
                            acc = accs[c * 4 + jj]
                            nc.scalar.activation(
                                dst[:, jt, cs], acc[:],
                                mybir.ActivationFunctionType.Relu,
                                bias=btile[:, l, jt:jt + 1],
                            )
                    if l == N_LEVELS - 1:
                        # final level: dst == actA; ship this jg's four
                        # feature blocks via SWDGE (GpSimd) so the store
                        # never head-of-line-blocks the paced W chain.
                        # Last jg goes out per chunk so the tail DMA is
                        # small.
                        if jg < JG - 1:
                            nc.gpsimd.dma_start(
                                outv[:, jg * 4:(jg + 1) * 4, :].bitcast(F32R),
                                dst[:, jg * 4:(jg + 1) * 4, :])
                        else:
                            for c in range(nchunks):
                                cs = slice(c * chunk, (c + 1) * chunk)
                                for jj in range(4):
                                    jt = jg * 4 + jj
                                    last = (c == nchunks - 1 and jj == 3)
                                    eng = nc.scalar if last else nc.gpsimd
                                    eng.dma_start(
                                        outv[:, jt, cs].bitcast(F32R),
                                        dst[:, jt, cs])

    nc.compile()
    _cache[key] = nc
    return nc


def _linearize_w(W: np.ndarray) -> np.ndarray:
    """[D, D] -> [JG, 128, KT*512] with (jg, p, kt, jc) = W[kt*128+p, jg*512+jc]."""
    return np.ascontiguousarray(
        W.reshape(KT, 128, JG, 512).transpose(2, 1, 0, 3).reshape(
            JG, 128, KT * 512))


def _plan(path_mask: np.ndarray):
    pm = np.asarray(path_mask)
    e3 = (pm[:, 0] * 4 + pm[:, 1] * 2 + pm[:, 2]).astype(np.int64)
    counts = np.bincount(e3, minlength=N_CORES)
    maxg = int(max(counts.max(), 1))
    nchunks = max(1, math.ceil(maxg / 512))
    chunk = max(256, math.ceil(maxg / nchunks))
    chunk = min(512, (chunk + 7) // 8 * 8)
    Bc = chunk * nchunks
    return e3, maxg, Bc, chunk, nchunks


def kernel(x, path_mask, W0, b0, W1, b1, W2, b2, W3, b3, _trace=False):
    x = np.ascontiguousarray(np.asarray(x, dtype=np.float32))
    Wls = [np.asarray(W, dtype=np.float32) for W in (W0, W1, W2, W3)]
    bls = [np.asarray(b, dtype=np.float32) for b in (b0, b1, b2, b3)]
    B = x.shape[0]

    e3, maxg, Bc, chunk, nchunks = _plan(path_mask)
    if Bc > 672:
        # extreme routing skew: SBUF can't hold the activations in one
        # pass; fall back to multiple 512-row passes per core.
        Bc, chunk, nchunks = 512, 512, 1
    nseg = math.ceil(maxg / Bc)
    nc = _build(Bc, chunk, nchunks)

    core_rows = [np.nonzero(e3 == c)[0] for c in range(N_CORES)]
    wb_maps = []
    for c in range(N_CORES):
        eids = (0, c >> 2, c >> 1, c)
        wb_maps.append({
            **{f"W{l}": _linearize_w(Wls[l][eids[l]])
               for l in range(N_LEVELS)},
            "bias": np.ascontiguousarray(
                np.stack([bls[l][eids[l]] for l in range(N_LEVELS)])),
        })

    out_full = np.zeros((B, D), dtype=np.float32)
    last_res = None
    for s in range(nseg):
        in_maps = []
        for c in range(N_CORES):
            rows = core_rows[c][s * Bc:(s + 1) * Bc]
            xTc = np.zeros((D, Bc), dtype=np.float32)
            xTc[:, :len(rows)] = x[rows].T
            in_maps.append({"xT": xTc, **wb_maps[c]})
        res = run_bass_kernel_spmd(nc, in_maps, list(range(N_CORES)),
                                   trace=_trace)
        last_res = res
        for c in range(N_CORES):
            rows = core_rows[c][s * Bc:(s + 1) * Bc]
            out_full[rows] = res.results[c]["out"][:, :len(rows)].T
    if _trace:
        return out_full, last_res
    return out_full



# revision 2
# speedup vs baseline: 1.1178x; 1.1178x over previous
"""Binary-tree gated-expert MoE kernel for 8 Trainium2 NeuronCores.

Reference computation (B=4096, D=2048, 4 levels, 1/2/4/8 experts):
    h = x
    for level l: h = relu(h @ Wl[eid_l] + bl[eid_l])
where eid_l is the l-bit prefix of the 3-bit leaf id built from
path_mask[:, 0:3].

Strategy: expert-parallel over the 8 leaves with host-side dispatch.
Sorting samples by leaf id makes every level's expert groups contiguous
(level-l ids are prefixes of the leaf id), so core c processes leaf
group c and needs exactly 4 weight matrices: W0[0], W1[c>>2], W2[c>>1],
W3[c].  Groups are Binomial(B, 1/8) ~ 512+-21 rows; each is padded to a
common per-core batch Bc.  On device each core runs 4 chained
matmul+relu levels in feature-major layout (activations stay transposed
[D, Bc] in SBUF across all levels; output partition dim = output
features, so no transposes anywhere).

Everything streams in bf16 (weights, x, activations, output) with fp32
PSUM accumulation: vs the fp32r variant this halves HBM traffic
(~255 GB/s/core -> ~130, well under the 358 GB/s/core ceiling) and
enables the PE's fast-weight-load path so the per-matmul LDWEIGHTS
(~53ns) hides completely under the 113ns matmuls.  Weights stream per
256-feature j-group (8 per level), so only 4 of the 8 PSUM banks are
tied up per group and group/level boundaries pipeline without PE gaps.
The two 272-column PSUM chunks are emitted back-to-back per weight tile
so the stationary operand is loaded once per (k-tile, j-block).
"""

import math

import numpy as np
import ml_dtypes

from concourse import bacc, mybir, tile
from concourse.bass_utils import run_bass_kernel_spmd

D = 2048
KT = D // 128          # 16 contraction k-tiles
JT = D // 128          # 16 output-feature blocks
JW = 2                 # j-blocks (256 features) per weight group
JG = JT // JW          # 8 weight groups per level
N_CORES = 8
N_LEVELS = 4
F32 = mybir.dt.float32
BF16 = mybir.dt.bfloat16
NPBF16 = ml_dtypes.bfloat16

_cache: dict = {}


def _build(Bc: int, chunk: int, nchunks: int):
    """Build + compile the per-core Bass program for batch Bc = chunk*nchunks."""
    key = (Bc, chunk, nchunks)
    if key in _cache:
        return _cache[key]

    nc = bacc.Bacc("TRN2", target_bir_lowering=False, debug=False,
                   num_devices=N_CORES)

    # Weights arrive host-linearized as [JG, 128, KT*256]:
    # element (jg, p, kt, jc) = W[kt*128 + p, jg*256 + jc], so each DMA
    # reads long contiguous runs per partition.
    xT = nc.dram_tensor("xT", [D, Bc], BF16, kind="ExternalInput")
    Ws = [nc.dram_tensor(f"W{l}", [JG, 128, KT * 256], BF16,
                         kind="ExternalInput")
          for l in range(N_LEVELS)]
    bias = nc.dram_tensor("bias", [N_LEVELS, D], F32, kind="ExternalInput")
    out = nc.dram_tensor("out", [D, Bc], BF16, kind="ExternalOutput")

    xTv = xT.rearrange("(kt p) b -> p kt b", p=128)
    outv = out.rearrange("(jt p) b -> p jt b", p=128)
    bv = bias.rearrange("l (jt p) -> p l jt", p=128)
    NQ = 4                      # W DMA split: 4 quarters of 4 k-tiles
    KQ = KT // NQ               # k-tiles per quarter
    QW = KQ * JW * 128          # W free-dim elements per quarter
    PACE_WIN = 3                # max in-flight paced DMAs on the SP ring

    with tile.TileContext(nc) as tc:
        with (
            tc.tile_pool(name="acts", bufs=1) as acts,
            tc.tile_pool(name="w", bufs=3) as wpool,
            tc.tile_pool(name="ps", bufs=8, space="PSUM") as ps,
            tc.tile_pool(name="misc", bufs=1) as misc,
        ):
            actA = acts.tile([128, KT, Bc], BF16, tag="A")
            actB = acts.tile([128, KT, Bc], BF16, tag="B")
            btile = misc.tile([128, N_LEVELS, JT], F32)
            nc.scalar.dma_start(btile[:], bv)

            # Warm the PE HAM clock gate during the DMA lead-in: ~4us of
            # throwaway matmuls on a zeroed tile so the first real matmul
            # runs at 2.4GHz instead of 1.2GHz.  (~10 cold N=512 matmuls
            # cover the 3.4us HAM window; more than that just delays the
            # real work.)
            warm = misc.tile([128, 512], BF16)
            nc.gpsimd.memset(warm[:], 0.0)
            wacc = ps.tile([128, 512], F32, tag="ps", name="wacc")
            for _ in range(10):
                nc.tensor.matmul(wacc[:], warm[:, :128], warm[:],
                                 start=True, stop=True)

            # All bulk input DMAs go on the SP ring, chained so at most
            # PACE_WIN are in flight.  The HW SDMA engines round-robin
            # packets across every queued transfer, so an unbounded
            # backlog makes every transfer finish near the end; a short
            # chain keeps completion order = consumption order with the
            # stream still running at full HBM rate.
            paced = []

            def paced_dma(dst_ap, src_ap):
                h = nc.sync.dma_start(dst_ap, src_ap)
                # tighter window while the first matmul's inputs stream
                win = 2 if len(paced) < 4 else PACE_WIN
                if len(paced) >= win:
                    tile.add_dep_helper(h.ins, paced[-win].ins,
                                        reason="dma pacing chain")
                paced.append(h)
                return h

            # x pieces, emitted interleaved with the first weight
            # quarters in consumption-priority order.  The first piece
            # (k-quarter 0, chunk 0) is all the first matmul needs.
            pend_x = []
            if nchunks > 1:
                pend_x.append((slice(0, KQ), slice(chunk, Bc)))
            for q in range(1, NQ):
                pend_x.append((slice(q * KQ, (q + 1) * KQ), slice(0, Bc)))
            paced_dma(actA[:, 0:KQ, 0:chunk], xTv[:, 0:KQ, 0:chunk])

            for l in range(N_LEVELS):
                src = actA if l % 2 == 0 else actB
                dst = actB if l % 2 == 0 else actA
                for jg in range(JG):
                    wt = wpool.tile([128, KT, JW * 128], BF16, tag="w")
                    wflat = wt.rearrange("p kt j -> p (kt j)")
                    accs = [ps.tile([128, chunk], F32, tag="ps", name="acc")
                            for _ in range(nchunks * JW)]
                    for q in range(NQ):
                        paced_dma(
                            wflat[:, q * QW:(q + 1) * QW],
                            Ws[l][jg][:, q * QW:(q + 1) * QW])
                        if l == 0 and pend_x:
                            ks, cs_x = pend_x.pop(0)
                            paced_dma(actA[:, ks, cs_x], xTv[:, ks, cs_x])
                        for kt in range(q * KQ, (q + 1) * KQ):
                            for jj in range(JW):
                                # chunk-inner: both chunks reuse the
                                # freshly loaded stationary weights
                                for c in range(nchunks):
                                    cs = slice(c * chunk, (c + 1) * chunk)
                                    nc.tensor.matmul(
                                        accs[c * JW + jj][:],
                                        wt[:, kt, jj * 128:(jj + 1) * 128],
                                        src[:, kt, cs],
                                        start=(kt == 0),
                                        stop=(kt == KT - 1),
                                    )
                    for c in range(nchunks):
                        cs = slice(c * chunk, (c + 1) * chunk)
                        for jj in range(JW):
                            jt = jg * JW + jj
                            acc = accs[c * JW + jj]
                            nc.scalar.activation(
                                dst[:, jt, cs], acc[:],
                                mybir.ActivationFunctionType.Relu,
                                bias=btile[:, l, jt:jt + 1],
                            )
                    if l == N_LEVELS - 1:
                        # final level: dst == actA; ship this jg's two
                        # feature blocks via SWDGE (GpSimd) so the store
                        # never head-of-line-blocks the paced W chain.
                        # Last jg goes out per chunk so the tail DMA is
                        # small.
                        if jg < JG - 1:
                            nc.gpsimd.dma_start(
                                outv[:, jg * JW:(jg + 1) * JW, :],
                                dst[:, jg * JW:(jg + 1) * JW, :])
                        else:
                            for c in range(nchunks):
                                cs = slice(c * chunk, (c + 1) * chunk)
                                for jj in range(JW):
                                    jt = jg * JW + jj
                                    last = (c == nchunks - 1 and jj == JW - 1)
                                    eng = nc.scalar if last else nc.gpsimd
                                    eng.dma_start(
                                        outv[:, jt, cs],
                                        dst[:, jt, cs])

    nc.compile()
    _cache[key] = nc
    return nc


def _linearize_w(W: np.ndarray) -> np.ndarray:
    """[D, D] -> [JG, 128, KT*JW*128] bf16 with
    (jg, p, kt, jc) = W[kt*128+p, jg*JW*128+jc]."""
    return np.ascontiguousarray(
        W.reshape(KT, 128, JG, JW * 128).transpose(2, 1, 0, 3).reshape(
            JG, 128, KT * JW * 128).astype(NPBF16))


def _plan(path_mask: np.ndarray):
    pm = np.asarray(path_mask)
    e3 = (pm[:, 0] * 4 + pm[:, 1] * 2 + pm[:, 2]).astype(np.int64)
    counts = np.bincount(e3, minlength=N_CORES)
    maxg = int(max(counts.max(), 1))
    nchunks = max(1, math.ceil(maxg / 512))
    chunk = max(256, math.ceil(maxg / nchunks))
    chunk = min(512, (chunk + 7) // 8 * 8)
    Bc = chunk * nchunks
    return e3, maxg, Bc, chunk, nchunks


def kernel(x, path_mask, W0, b0, W1, b1, W2, b2, W3, b3, _trace=False):
    x = np.ascontiguousarray(np.asarray(x, dtype=np.float32))
    Wls = [np.asarray(W, dtype=np.float32) for W in (W0, W1, W2, W3)]
    bls = [np.asarray(b, dtype=np.float32) for b in (b0, b1, b2, b3)]
    B = x.shape[0]

    e3, maxg, Bc, chunk, nchunks = _plan(path_mask)
    if Bc > 1536:
        # extreme routing skew: SBUF can't hold the activations in one
        # pass; fall back to multiple 512-row passes per core.
        Bc, chunk, nchunks = 512, 512, 1
    nseg = math.ceil(maxg / Bc)
    nc = _build(Bc, chunk, nchunks)

    core_rows = [np.nonzero(e3 == c)[0] for c in range(N_CORES)]
    # linearize each needed expert matrix once (cores share references)
    lin: dict = {}
    for c in range(N_CORES):
        eids = (0, c >> 2, c >> 1, c)
        for l in range(N_LEVELS):
            if (l, eids[l]) not in lin:
                lin[(l, eids[l])] = _linearize_w(Wls[l][eids[l]])
    wb_maps = []
    for c in range(N_CORES):
        eids = (0, c >> 2, c >> 1, c)
        wb_maps.append({
            **{f"W{l}": lin[(l, eids[l])] for l in range(N_LEVELS)},
            "bias": np.ascontiguousarray(
                np.stack([bls[l][eids[l]] for l in range(N_LEVELS)])),
        })

    xT_bf16 = x.T.astype(NPBF16)
    out_full = np.zeros((B, D), dtype=np.float32)
    last_res = None
    for s in range(nseg):
        in_maps = []
        for c in range(N_CORES):
            rows = core_rows[c][s * Bc:(s + 1) * Bc]
            xTc = np.zeros((D, Bc), dtype=NPBF16)
            xTc[:, :len(rows)] = xT_bf16[:, rows]
            in_maps.append({"xT": xTc, **wb_maps[c]})
        res = run_bass_kernel_spmd(nc, in_maps, list(range(N_CORES)),
                                   trace=_trace)
        last_res = res
        for c in range(N_CORES):
            rows = core_rows[c][s * Bc:(s + 1) * Bc]
            out_full[rows] = res.results[c]["out"][:, :len(rows)].T.astype(
                np.float32)
    if _trace:
        return out_full, last_res
    return out_full


# revision 6
# speedup vs baseline: 1.1208x; 1.0027x over previous
"""Binary-tree gated-expert MoE kernel for 8 Trainium2 NeuronCores.

Reference computation (B=4096, D=2048, 4 levels, 1/2/4/8 experts):
    h = x
    for level l: h = relu(h @ Wl[eid_l] + bl[eid_l])
where eid_l is the l-bit prefix of the 3-bit leaf id built from
path_mask[:, 0:3].

Strategy: expert-parallel over the 8 leaves with host-side dispatch.
Sorting samples by leaf id makes every level's expert groups contiguous
(level-l ids are prefixes of the leaf id), so core c processes leaf
group c and needs exactly 4 weight matrices: W0[0], W1[c>>2], W2[c>>1],
W3[c].  Groups are Binomial(B, 1/8) ~ 512+-21 rows; each is padded to a
common per-core batch Bc.  On device each core runs 4 chained
matmul+relu levels in feature-major layout (activations stay transposed
[D, Bc] in SBUF across all levels; output partition dim = output
features, so no transposes anywhere).

Everything streams in bf16 (weights, x, activations, output) with fp32
PSUM accumulation: vs the fp32r variant this halves HBM traffic
(~255 GB/s/core -> ~130, well under the 358 GB/s/core ceiling) and
enables the PE's fast-weight-load path so the per-matmul LDWEIGHTS
(~53ns) hides completely under the 113ns matmuls.  Weights stream per
256-feature j-group (8 per level), so only 4 of the 8 PSUM banks are
tied up per group and group/level boundaries pipeline without PE gaps.
The two 272-column PSUM chunks are emitted back-to-back per weight tile
so the stationary operand is loaded once per (k-tile, j-block).
"""

import math

import numpy as np
import ml_dtypes

from concourse import bacc, mybir, tile
from concourse.bass_utils import run_bass_kernel_spmd

D = 2048
KT = D // 128          # 16 contraction k-tiles
JT = D // 128          # 16 output-feature blocks
JW = 2                 # j-blocks (256 features) per weight group
JG = JT // JW          # 8 weight groups per level
N_CORES = 8
N_LEVELS = 4
F32 = mybir.dt.float32
BF16 = mybir.dt.bfloat16
NPBF16 = ml_dtypes.bfloat16

_cache: dict = {}


def _build(Bc: int, chunk: int, nchunks: int):
    """Build + compile the per-core Bass program for batch Bc = chunk*nchunks."""
    key = (Bc, chunk, nchunks)
    if key in _cache:
        return _cache[key]

    nc = bacc.Bacc("TRN2", target_bir_lowering=False, debug=False,
                   num_devices=N_CORES)

    # Weights arrive host-linearized as [JG, 128, KT*256]:
    # element (jg, p, kt, jc) = W[kt*128 + p, jg*256 + jc], so each DMA
    # reads long contiguous runs per partition.
    xT = nc.dram_tensor("xT", [D, Bc], BF16, kind="ExternalInput")
    Ws = [nc.dram_tensor(f"W{l}", [JG, 128, KT * 256], BF16,
                         kind="ExternalInput")
          for l in range(N_LEVELS)]
    bias = nc.dram_tensor("bias", [N_LEVELS, D], F32, kind="ExternalInput")
    out = nc.dram_tensor("out", [D, Bc], BF16, kind="ExternalOutput")

    xTv = xT.rearrange("(kt p) b -> p kt b", p=128)
    outv = out.rearrange("(jt p) b -> p jt b", p=128)
    bv = bias.rearrange("l (jt p) -> p l jt", p=128)
    NQ = 4                      # W DMA split: 4 quarters of 4 k-tiles
    KQ = KT // NQ               # k-tiles per quarter
    QW = KQ * JW * 128          # W free-dim elements per quarter
    PACE_WIN = 3                # max in-flight paced DMAs on the SP ring

    with tile.TileContext(nc) as tc:
        with (
            tc.tile_pool(name="acts", bufs=1) as acts,
            tc.tile_pool(name="w", bufs=3) as wpool,
            tc.tile_pool(name="ps", bufs=8, space="PSUM") as ps,
            tc.tile_pool(name="misc", bufs=1) as misc,
        ):
            actA = acts.tile([128, KT, Bc], BF16, tag="A")
            actB = acts.tile([128, KT, Bc], BF16, tag="B")
            btile = misc.tile([128, N_LEVELS, JT], F32)
            nc.scalar.dma_start(btile[:], bv)

            # Warm the PE HAM clock gate during the DMA lead-in: ~4us of
            # throwaway matmuls on a zeroed tile so the first real matmul
            # runs at 2.4GHz instead of 1.2GHz.  (~10 cold N=512 matmuls
            # cover the 3.4us HAM window; more than that just delays the
            # real work.)
            warm = misc.tile([128, 512], BF16)
            nc.gpsimd.memset(warm[:], 0.0)
            wacc = ps.tile([128, 512], F32, tag="ps", name="wacc")
            for _ in range(6):
                nc.tensor.matmul(wacc[:], warm[:, :128], warm[:],
                                 start=True, stop=True)

            # All bulk input DMAs go on the SP ring, chained so at most
            # PACE_WIN are in flight.  The HW SDMA engines round-robin
            # packets across every queued transfer, so an unbounded
            # backlog makes every transfer finish near the end; a short
            # chain keeps completion order = consumption order with the
            # stream still running at full HBM rate.
            paced = []

            def paced_dma(dst_ap, src_ap):
                h = nc.sync.dma_start(dst_ap, src_ap)
                # tighter window while the first matmul's inputs stream
                win = 2 if len(paced) < 2 else PACE_WIN
                if len(paced) >= win:
                    tile.add_dep_helper(h.ins, paced[-win].ins,
                                        reason="dma pacing chain")
                paced.append(h)
                return h

            # x pieces in exact consumption order (k-quarter major,
            # chunk minor), interleaved 2:1 with the first weight
            # quarters so the whole x tensor lands within jg0's matmul
            # window.  The first piece (k-quarter 0, chunk 0) is all the
            # first matmul needs.
            pend_x = []
            for q in range(NQ):
                for c in range(nchunks):
                    pend_x.append((slice(q * KQ, (q + 1) * KQ),
                                   slice(c * chunk, (c + 1) * chunk)))
            ks, cs_x = pend_x.pop(0)
            paced_dma(actA[:, ks, cs_x], xTv[:, ks, cs_x])

            for l in range(N_LEVELS):
                src = actA if l % 2 == 0 else actB
                dst = actB if l % 2 == 0 else actA
                for jg in range(JG):
                    wt = wpool.tile([128, KT, JW * 128], BF16, tag="w")
                    wflat = wt.rearrange("p kt j -> p (kt j)")
                    accs = [ps.tile([128, chunk], F32, tag="ps", name="acc")
                            for _ in range(nchunks * JW)]
                    for q in range(NQ):
                        paced_dma(
                            wflat[:, q * QW:(q + 1) * QW],
                            Ws[l][jg][:, q * QW:(q + 1) * QW])
                        for _ in range(2):
                            if l == 0 and pend_x:
                                ks, cs_x = pend_x.pop(0)
                                paced_dma(actA[:, ks, cs_x],
                                          xTv[:, ks, cs_x])
                        for kt in range(q * KQ, (q + 1) * KQ):
                            for jj in range(JW):
                                # chunk-inner: both chunks reuse the
                                # freshly loaded stationary weights
                                for c in range(nchunks):
                                    cs = slice(c * chunk, (c + 1) * chunk)
                                    nc.tensor.matmul(
                                        accs[c * JW + jj][:],
                                        wt[:, kt, jj * 128:(jj + 1) * 128],
                                        src[:, kt, cs],
                                        start=(kt == 0),
                                        stop=(kt == KT - 1),
                                    )
                    tail = l == N_LEVELS - 1 and jg == JG - 1
                    for c in range(nchunks):
                        cs = slice(c * chunk, (c + 1) * chunk)
                        for jj in range(JW):
                            jt = jg * JW + jj
                            acc = accs[c * JW + jj]
                            if tail and jj % 2 == 1:
                                # split the program-tail relus across two
                                # engines so they don't serialize on ACT
                                nc.vector.tensor_scalar(
                                    dst[:, jt, cs], acc[:],
                                    btile[:, l, jt:jt + 1], 0.0,
                                    op0=mybir.AluOpType.add,
                                    op1=mybir.AluOpType.max,
                                )
                            else:
                                nc.scalar.activation(
                                    dst[:, jt, cs], acc[:],
                                    mybir.ActivationFunctionType.Relu,
                                    bias=btile[:, l, jt:jt + 1],
                                )
                    if l == N_LEVELS - 1:
                        # final level: dst == actA; ship this jg's two
                        # feature blocks via SWDGE (GpSimd) so the store
                        # never head-of-line-blocks the paced W chain.
                        # Last jg goes out per (chunk, jt) tile on four
                        # different trigger queues so the tail transfers
                        # overlap.
                        if jg < JG - 1:
                            nc.gpsimd.dma_start(
                                outv[:, jg * JW:(jg + 1) * JW, :],
                                dst[:, jg * JW:(jg + 1) * JW, :])
                        else:
                            engs = [nc.gpsimd, nc.gpsimd, nc.sync, nc.scalar]
                            for c in range(nchunks):
                                cs = slice(c * chunk, (c + 1) * chunk)
                                for jj in range(JW):
                                    jt = jg * JW + jj
                                    eng = engs[(c * JW + jj) % 4]
                                    eng.dma_start(
                                        outv[:, jt, cs],
                                        dst[:, jt, cs])

    nc.compile()
    _cache[key] = nc
    return nc


def _linearize_w(W: np.ndarray) -> np.ndarray:
    """[D, D] -> [JG, 128, KT*JW*128] bf16 with
    (jg, p, kt, jc) = W[kt*128+p, jg*JW*128+jc]."""
    return np.ascontiguousarray(
        W.reshape(KT, 128, JG, JW * 128).transpose(2, 1, 0, 3).reshape(
            JG, 128, KT * JW * 128).astype(NPBF16))


def _plan(path_mask: np.ndarray):
    pm = np.asarray(path_mask)
    e3 = (pm[:, 0] * 4 + pm[:, 1] * 2 + pm[:, 2]).astype(np.int64)
    counts = np.bincount(e3, minlength=N_CORES)
    maxg = int(max(counts.max(), 1))
    nchunks = max(1, math.ceil(maxg / 512))
    chunk = max(256, math.ceil(maxg / nchunks))
    chunk = min(512, (chunk + 7) // 8 * 8)
    Bc = chunk * nchunks
    return e3, maxg, Bc, chunk, nchunks


def kernel(x, path_mask, W0, b0, W1, b1, W2, b2, W3, b3, _trace=False):
    x = np.ascontiguousarray(np.asarray(x, dtype=np.float32))
    Wls = [np.asarray(W, dtype=np.float32) for W in (W0, W1, W2, W3)]
    bls = [np.asarray(b, dtype=np.float32) for b in (b0, b1, b2, b3)]
    B = x.shape[0]

    e3, maxg, Bc, chunk, nchunks = _plan(path_mask)
    if Bc > 1536:
        # extreme routing skew: SBUF can't hold the activations in one
        # pass; fall back to multiple 512-row passes per core.
        Bc, chunk, nchunks = 512, 512, 1
    nseg = math.ceil(maxg / Bc)
    nc = _build(Bc, chunk, nchunks)

    core_rows = [np.nonzero(e3 == c)[0] for c in range(N_CORES)]
    # linearize each needed expert matrix once (cores share references)
    lin: dict = {}
    for c in range(N_CORES):
        eids = (0, c >> 2, c >> 1, c)
        for l in range(N_LEVELS):
            if (l, eids[l]) not in lin:
                lin[(l, eids[l])] = _linearize_w(Wls[l][eids[l]])
    wb_maps = []
    for c in range(N_CORES):
        eids = (0, c >> 2, c >> 1, c)
        wb_maps.append({
            **{f"W{l}": lin[(l, eids[l])] for l in range(N_LEVELS)},
            "bias": np.ascontiguousarray(
                np.stack([bls[l][eids[l]] for l in range(N_LEVELS)])),
        })

    xT_bf16 = x.T.astype(NPBF16)
    out_full = np.zeros((B, D), dtype=np.float32)
    last_res = None
    for s in range(nseg):
        in_maps = []
        for c in range(N_CORES):
            rows = core_rows[c][s * Bc:(s + 1) * Bc]
            xTc = np.zeros((D, Bc), dtype=NPBF16)
            xTc[:, :len(rows)] = xT_bf16[:, rows]
            in_maps.append({"xT": xTc, **wb_maps[c]})
        res = run_bass_kernel_spmd(nc, in_maps, list(range(N_CORES)),
                                   trace=_trace)
        last_res = res
        for c in range(N_CORES):
            rows = core_rows[c][s * Bc:(s + 1) * Bc]
            out_full[rows] = res.results[c]["out"][:, :len(rows)].T.astype(
                np.float32)
    if _trace:
        return out_full, last_res
    return out_full
